# revision 58
# baseline (speedup 1.0000x reference)
"""Trainium2 Bass kernel for a cross-attention + adaLN-modulated-LN + linear block.

Sharding: 8 cores = 4 batches (B) x 2 token-halves of S=4096.  No collectives:
each core recomputes the (small) kv projection for its batch and processes all
16 attention heads for its 2048 tokens, then LN/modulation/final-linear for its
8 frames.  Host slices inputs per core and reassembles the output.

Device layout is feature-major ("transposed"): activations live as [C, tok]
tiles so every matmul contracts over the SBUF partition dim.

Precision plan (tolerance 2e-2; the residual damps attention-path noise ~40x,
measured end-to-end rel err ~3.9e-3):
  - q / kv / proj projections run in fp8e5m2 with perf_mode=DoubleRow (2
    contraction rows per PE cell -> half the matmul time).  The fp8 rhs
    operands are the natural [128, 2, N] slices of feature-major tiles, so no
    device-side re-layout is needed; weights are host-interleaved.
  - attention operands (kT, q, exp(scores), vv) are bf16; PSUM accumulation
    is fp32 throughout.
  - the LN-stats / final-linear path stays bf16/fp32 (output precision).
Softmax: scores for 2 key-blocks land side-by-side in one 2-bank PSUM tile so
exp runs as one [128,1024] Activation op (the Activation engine is the
attention-phase bottleneck); the softmax denominator rides the attention-output
matmul as an all-ones lhsT column, and each head's 1/denom row is broadcast
across partitions with gpsimd partition_broadcast (NOTE: its output tile must
be base-partition 0; the odd head broadcasts to 128 rows and slices [64:128]).
Scheduling notes: DMAs only on the SP/Act hardware-DGE queues (gpsimd DMA is
software-DGE and stalls the Pool engine); input loads are few and large with
tc.high_priority ordering v -> wkvv -> wkvk -> x -> wq; adaLN runs in the
post-LN phase so its streamed weight loads never block the PE queue; PSUM is
partitioned psT(3)+psV(5) in setup and sc(4)+ao(2)+psC(2) in attention, with
proj and its eviction overlapping attention via the psC pool.
"""

import sys

for _p in ("/opt/trn_rl_repo", "/opt/pypackages"):
    if _p not in sys.path:
        sys.path.append(_p)

import numpy as np
import ml_dtypes

import concourse.bacc as bacc
import concourse.tile as tile
from concourse import mybir
from concourse.bass_utils import run_bass_kernel_spmd
from concourse.masks import make_identity

FP = mybir.dt.float32
FPR = mybir.dt.float32r
BF = mybir.dt.bfloat16
F8 = mybir.dt.float8e5
AF = mybir.ActivationFunctionType
OP = mybir.AluOpType
PM = mybir.MatmulPerfMode
NP8 = ml_dtypes.float8_e5m2


def _r(ap):
    """View an fp32 AP as float32r for full-rate PE matmuls (same bits)."""
    return ap.bitcast(FPR)


# Problem sizes (hardcoded per spec).
B = 4
S = 4096
C = 1024
N2 = 512
H = 16
D = 64
T = 16
NT = 256          # tokens per frame
OUTD = 32

STOK = S // 2     # tokens per core
F = 8             # frames per core
G = C // 128      # 8 channel groups
GP = G // 2       # 4 channel-pair groups (DoubleRow)
TB = 512          # token block (matmul N)
NTB = STOK // TB  # 4
KB = N2 // 128    # 4 key blocks
SCALE = D ** -0.5
EPS = 1e-6
P = 128
SEG = 192         # vv cols per head-pair segment
DEBUG_DUMPS = False


def _body(nc, tc, io):
    with nc.allow_low_precision("bf16/fp8 matmul operands"):
        _body_inner(nc, tc, io)


def _body_inner(nc, tc, io):
    x, v, tvec, cmat = io["x_sl"], io["v_b"], io["t_b"], io["c_sl"]
    wq8, bq = io["wq8"], io["bq"]
    wkvk8, wkvv8, bkv = io["wkvk8"], io["wkvv8"], io["bkv"]
    wproj8, bproj = io["wproj8"], io["bproj"]
    wada_b, bada = io["wada_b"], io["bada"]
    wlin, blin = io["wlin"], io["blin"]
    yT = io["yT"]

    with (
        tc.tile_pool(name="consts", bufs=1) as consts,
        tc.tile_pool(name="xT", bufs=1) as xTp,
        tc.tile_pool(name="x8", bufs=1) as x8p,
        tc.tile_pool(name="qa", bufs=1) as qap,
        tc.tile_pool(name="qa8", bufs=1) as qa8p,
        tc.tile_pool(name="kT", bufs=1) as kTp,
        tc.tile_pool(name="wp", bufs=3) as wp,
        tc.tile_pool(name="wpj", bufs=1) as wpjp,
    ):
        # ---- constants / small inputs ----
        scratch = consts.tile([P, P], FP, tag="scratch")
        make_identity(nc, scratch)
        ident = consts.tile([P, P], FP, tag="ident")
        nc.vector.tensor_copy(out=_r(ident), in_=scratch)
        ones_t = consts.tile([P, P], FP, tag="ones")
        nc.vector.tensor_scalar(
            _r(ones_t), scratch, 0.0, 1.0, op0=OP.mult, op1=OP.add
        )
        ones_bf = consts.tile([P, P], BF, tag="onesbf")
        nc.vector.tensor_copy(out=ones_bf, in_=ones_t)
        eps_t = consts.tile([P, 1], FP, tag="eps")
        nc.vector.memset(eps_t, EPS)

        bq_t = consts.tile([P, G], FP, tag="bq")
        nc.sync.dma_start(out=bq_t, in_=bq.ap().rearrange("(g p) -> p g", p=P))
        bkvk_t = consts.tile([P, G], FP, tag="bkvk")
        nc.sync.dma_start(
            out=bkvk_t, in_=bkv.ap()[0:C].rearrange("(g p) -> p g", p=P)
        )
        bproj_t = consts.tile([P, G], FP, tag="bproj")
        nc.sync.dma_start(out=bproj_t, in_=bproj.ap().rearrange("(g p) -> p g", p=P))
        bada_t = consts.tile([P, 16], FP, tag="bada")
        nc.sync.dma_start(out=bada_t, in_=bada.ap().rearrange("(g p) -> p g", p=P))
        blin_row = consts.tile([1, OUTD], FP, tag="blin")
        nc.sync.dma_start(
            out=blin_row, in_=blin.ap().rearrange("(one o) -> one o", one=1)
        )
        t_t = consts.tile([P, G], FP, tag="tvec")
        nc.sync.dma_start(out=t_t, in_=tvec.ap().rearrange("(g p) -> p g", p=P))
        wlin_sb = consts.tile([P, G, OUTD], FP, tag="wlin")
        nc.sync.dma_start(
            out=_r(wlin_sb), in_=_r(wlin.ap().rearrange("(ci p) o -> p ci o", p=P))
        )
        silu_t = consts.tile([P, G, F], BF, tag="silu")
        ada_t = consts.tile([P, 16, F], FP, tag="ada")
        bkvv_row = consts.tile([1, C], FP, tag="bkvv")
        nc.sync.dma_start(
            out=_r(bkvv_row),
            in_=_r(bkv.ap()[C : 2 * C].rearrange("(one n) -> one n", one=1)),
        )
        c_nat = consts.tile([F, C], FP, tag="cnat")
        nc.sync.dma_start(out=_r(c_nat), in_=_r(cmat[:, :]))

        # ---- persistent activation buffers ----
        xT_t = xTp.tile([P, G, STOK], BF, tag="xT")       # x then x1 (bf16)
        xT8_t = x8p.tile([P, G, STOK], F8, tag="xT8")     # fp8 copy for q-proj
        qa_t = qap.tile([P, G, STOK], BF, tag="qa")       # q (bf16)
        qa8_t = qa8p.tile([P, G, STOK], F8, tag="qa8")    # normalized attn out
        kt_t = kTp.tile([P, G, N2], BF, tag="kT")         # k (bf16)
        # proj weights stay resident (fp8: 8KB/partition); DMA issued late
        wpj_t = wpjp.tile([P, G, GP, 2, P], F8, tag="wpj")

        # vv: key-major value matrix (bf16). Per head-pair g, a SEG=192-col
        # segment: [0:64] even-head data, [64] ones column (denominator),
        # [65:128] junk, [128:192] odd-head data.  Even lhsT window cols 0:65
        # -> rows 0:64 data + row 64 denom; odd window cols 64:192 -> row 0
        # denom + rows 64:128 data.
        with tc.tile_pool(name="vv", bufs=1) as vvp:
            vv = [
                vvp.tile([P, 8 * SEG], BF, name=f"vv{kb}", tag=f"vv{kb}")
                for kb in range(KB)
            ]
            for kb in range(KB):
                ones_cols = vv[kb].rearrange("p (a r) -> p a r", r=SEG)[:, :, 64:65]
                nc.vector.memset(ones_cols, 1.0)

            with (
                tc.tile_pool(name="psT", bufs=3, space="PSUM") as psT,
                tc.tile_pool(name="psV", bufs=5, space="PSUM") as psV,
            ):
                with (
                    tc.tile_pool(name="vT", bufs=1) as vTp,
                    tc.tile_pool(name="ld", bufs=4) as ldp,
                ):
                    vT8_t = vTp.tile([P, G, N2], F8, tag="vT8")

                    # ---- transpose v (evict straight to fp8) ----
                    for kt in range(KB):
                        v_nat = ldp.tile([P, C], FP, name="vn", tag="ldv", bufs=2)
                        with tc.high_priority(offset=100000):
                            for vh in range(2):
                                (nc.sync, nc.scalar)[vh].dma_start(
                                    out=_r(v_nat[:, vh * TB : (vh + 1) * TB]),
                                    in_=_r(v[kt * P : (kt + 1) * P, vh * TB : (vh + 1) * TB]),
                                )
                        for g4 in range(2):
                            pt = psT.tile([P, 4 * P], FP, name="ptv", tag="ps")
                            for j in range(4):
                                jc = g4 * TB + j * P
                                nc.tensor.transpose(
                                    _r(pt[:, j * P : (j + 1) * P]),
                                    _r(v_nat[:, jc : jc + P]),
                                    _r(ident),
                                )
                            nc.any.tensor_copy(
                                out=vT8_t[:, g4 * 4 : g4 * 4 + 4, kt * P : (kt + 1) * P],
                                in_=pt.rearrange("p (a c) -> p a c", c=P),
                            )

                    # ---- transpose x (evict bf16 + fp8 via Act from psum) ----
                    for tt2 in range(STOK // (2 * P)):
                        x_nat = ldp.tile([P, 2, C], FP, name="xn", tag="ldx", bufs=2)
                        dma_eng = (nc.sync, nc.scalar)[tt2 % 2]
                        with tc.high_priority(offset=98000):
                            dma_eng.dma_start(
                                out=_r(x_nat),
                                in_=_r(
                                    x[2 * tt2 * P : (2 * tt2 + 2) * P, :].rearrange(
                                        "(u p) c -> p u c", p=P
                                    )
                                ),
                            )
                        for u in range(2):
                            tt = 2 * tt2 + u
                            for g4 in range(2):
                                pt = psT.tile([P, 4 * P], FP, name="ptx", tag="ps")
                                for j in range(4):
                                    jc = g4 * TB + j * P
                                    nc.tensor.transpose(
                                        _r(pt[:, j * P : (j + 1) * P]),
                                        _r(x_nat[:, u, jc : jc + P]),
                                        _r(ident),
                                    )
                                gsl = slice(g4 * 4, g4 * 4 + 4)
                                tsl = slice(tt * P, (tt + 1) * P)
                                if (tt + g4) % 2 == 0:
                                    nc.vector.tensor_copy(
                                        out=xT_t[:, gsl, tsl],
                                        in_=pt.rearrange("p (a c) -> p a c", c=P),
                                    )
                                    nc.scalar.copy(
                                        out=xT8_t[:, gsl, tsl],
                                        in_=pt.rearrange("p (a c) -> p a c", c=P),
                                    )
                                else:
                                    nc.scalar.copy(
                                        out=xT_t[:, gsl, tsl],
                                        in_=pt.rearrange("p (a c) -> p a c", c=P),
                                    )
                                    nc.vector.tensor_copy(
                                        out=xT8_t[:, gsl, tsl],
                                        in_=pt.rearrange("p (a c) -> p a c", c=P),
                                    )

                    # ---- vv build (fp8 DoubleRow), bias folded via ones-row MM ----
                    wvv_sb = ldp.tile([P, GP, 2, C], F8, name="wvvsb", tag="wv8", bufs=1)
                    with tc.high_priority(offset=99000):
                        nc.scalar.dma_start(
                            out=wvv_sb, in_=wkvv8.ap().rearrange("gp p j c -> p gp j c")
                        )
                    for half in range(2):
                        pss = [
                            psV.tile([P, TB], FP, name="psv", tag="ps")
                            for _ in range(KB)
                        ]
                        for kb in range(KB):
                            # bias first (fp32r), then fp8 DR accumulation
                            nc.tensor.matmul(
                                pss[kb],
                                lhsT=_r(ones_t[0:1, 0:P]),
                                rhs=_r(bkvv_row[0:1, half * TB : (half + 1) * TB]),
                                start=True,
                                stop=False,
                            )
                            for gp in range(GP):
                                nc.tensor.matmul(
                                    pss[kb],
                                    lhsT=vT8_t[:, 2 * gp : 2 * gp + 2, kb * P : (kb + 1) * P],
                                    rhs=wvv_sb[:, gp, :, half * TB : (half + 1) * TB],
                                    start=False,
                                    stop=(gp == GP - 1),
                                    perf_mode=PM.DoubleRow,
                                )
                        for kb in range(KB):
                            vvr = vv[kb].rearrange("p (a r) -> p a r", r=SEG)
                            src = pss[kb].rearrange("p (a q j) -> p a q j", q=2, j=64)
                            gs = slice(half * 4, half * 4 + 4)
                            eng = nc.vector if kb % 2 == 0 else nc.scalar
                            if kb % 2 == 0:
                                nc.vector.tensor_copy(
                                    out=vvr[:, gs, 0:64], in_=src[:, :, 0, :]
                                )
                                nc.vector.tensor_copy(
                                    out=vvr[:, gs, 128:192], in_=src[:, :, 1, :]
                                )
                            else:
                                nc.scalar.copy(out=vvr[:, gs, 0:64], in_=src[:, :, 0, :])
                                nc.scalar.copy(
                                    out=vvr[:, gs, 128:192], in_=src[:, :, 1, :]
                                )

                    # ---- kT (fp8 DoubleRow; frees vT afterwards) ----
                    wkv_sb = ldp.tile([P, G, GP, 2, P], F8, name="wkvsb", tag="wkvsb", bufs=1)
                    with tc.high_priority(offset=98500):
                        nc.scalar.dma_start(
                            out=wkv_sb,
                            in_=wkvk8.ap().rearrange("g gp p j m -> p g gp j m"),
                        )
                    for g in range(G):
                        psk = psV.tile([P, N2], FP, name="psk", tag="ps")
                        for gp in range(GP):
                            nc.tensor.matmul(
                                psk,
                                lhsT=wkv_sb[:, g, gp, :, :],
                                rhs=vT8_t[:, 2 * gp : 2 * gp + 2, :],
                                start=(gp == 0),
                                stop=(gp == GP - 1),
                                perf_mode=PM.DoubleRow,
                            )
                        nc.vector.tensor_scalar_add(
                            kt_t[:, g, :], psk, bkvk_t[:, g : g + 1]
                        )

                    # late prefetch of proj weights
                    nc.scalar.dma_start(
                        out=wpj_t,
                        in_=wproj8.ap().rearrange("cog gp p j m -> p cog gp j m"),
                    )


                # ---- q projection (fp8 DoubleRow; tb-major) ----
                wq_sb = wp.tile([P, G, GP, 2, P], F8, name="wqsb", tag="wqsb", bufs=1)
                with tc.high_priority(offset=97500):
                    nc.sync.dma_start(
                        out=wq_sb,
                        in_=wq8.ap().rearrange("cog gp p j m -> p cog gp j m"),
                    )
                for tb in range(NTB):
                    for cog in range(G):
                        pst = psV.tile([P, TB], FP, name="psq", tag="ps")
                        for gp in range(GP):
                            nc.tensor.matmul(
                                pst,
                                lhsT=wq_sb[:, cog, gp, :, :],
                                rhs=xT8_t[:, 2 * gp : 2 * gp + 2, tb * TB : (tb + 1) * TB],
                                start=(gp == 0),
                                stop=(gp == GP - 1),
                                perf_mode=PM.DoubleRow,
                            )
                        if (cog + tb) % 2 == 0:
                            nc.vector.tensor_scalar_add(
                                qa_t[:, cog, tb * TB : (tb + 1) * TB],
                                pst,
                                bq_t[:, cog : cog + 1],
                            )
                        else:
                            nc.scalar.activation(
                                out=qa_t[:, cog, tb * TB : (tb + 1) * TB],
                                in_=pst,
                                func=AF.Identity,
                                bias=bq_t[:, cog : cog + 1],
                                scale=1.0,
                            )

            # ---- attention (per head-pair; pipelined over (tb, g)) ----
            psC_cm = tc.tile_pool(name="psC", bufs=2, space="PSUM")
            psC = psC_cm.__enter__()
            with (
                tc.tile_pool(name="sc", bufs=2, space="PSUM") as scp,
                tc.tile_pool(name="ao", bufs=2, space="PSUM") as aop,
                tc.tile_pool(name="exp", bufs=6) as expp,
                tc.tile_pool(name="dn", bufs=2) as dnp,
            ):
                for tb in range(NTB):
                    tbs = slice(tb * TB, (tb + 1) * TB)
                    for g in range(G):
                        ao_tiles = [None, None]
                        rcps = [
                            dnp.tile([1, TB], BF, name=f"rcp{h2}", tag=f"rcp{h2}", bufs=3)
                            for h2 in range(2)
                        ]
                        for half in range(2):
                            r0 = half * 64
                            dr = 64 - 64 * half  # denom row: 64 (even), 0 (odd)
                            if half == 0:
                                lhs_lo, lhs_hi = 0, 65
                                ao_ps = aop.tile([65, TB], FP, name="aoe", tag="ao")
                            else:
                                lhs_lo, lhs_hi = 64, SEG
                                ao_ps = aop.tile([P, TB], FP, name="aoo", tag="ao")
                            ao_tiles[half] = ao_ps
                            exs = []
                            for kk in range(2):  # two fused (2-bank) score tiles
                                sc_ps = scp.tile([P, 2 * TB], FP, name="scs", tag="sc")
                                for kb2 in range(2):
                                    kb = 2 * kk + kb2
                                    nc.tensor.matmul(
                                        sc_ps[:, kb2 * TB : (kb2 + 1) * TB],
                                        lhsT=kt_t[r0 : r0 + 64, g, kb * P : (kb + 1) * P],
                                        rhs=qa_t[r0 : r0 + 64, g, tbs],
                                        start=True,
                                        stop=True,
                                    )
                                ex = expp.tile([P, 2 * TB], BF, tag="e")
                                nc.scalar.activation(
                                    out=ex, in_=sc_ps, func=AF.Exp, scale=SCALE
                                )
                                exs.append(ex)
                            for kb in range(KB):
                                nc.tensor.matmul(
                                    ao_ps,
                                    lhsT=vv[kb][
                                        :, g * SEG + lhs_lo : g * SEG + lhs_hi
                                    ],
                                    rhs=exs[kb // 2][:, (kb % 2) * TB : (kb % 2 + 1) * TB],
                                    start=(kb == 0),
                                    stop=(kb == KB - 1),
                                )
                            nc.vector.reciprocal(
                                out=rcps[half], in_=ao_ps[dr : dr + 1, :]
                            )
                        # broadcast each head's 1/denom on GpSimd.  NOTE:
                        # partition_broadcast requires a base-0 output tile;
                        # the odd head broadcasts to all 128 rows and the mul
                        # reads the base-64 slice.
                        dnb_e = dnp.tile([64, TB], BF, name="dnbe", tag="dne")
                        dnb_o = dnp.tile([P, TB], BF, name="dnbo", tag="dno")
                        nc.gpsimd.partition_broadcast(dnb_e, rcps[0])
                        nc.gpsimd.partition_broadcast(dnb_o, rcps[1])
                        nc.vector.tensor_mul(
                            qa8_t[0:64, g, tbs],
                            ao_tiles[0][0:64, :],
                            dnb_e,
                        )
                        nc.vector.tensor_mul(
                            qa8_t[64:128, g, tbs],
                            ao_tiles[1][64:128, :],
                            dnb_o[64:128, :],
                        )

        # ---- proj + residual (fp8 DoubleRow), then LN stats (shared psum) ----
        for tb in range(NTB):
            tbs = slice(tb * TB, (tb + 1) * TB)
            for cog in range(G):
                pst = psC.tile([P, TB], FP, name="psp", tag="ps", bufs=2)
                for gp in range(GP):
                    nc.tensor.matmul(
                        pst,
                        lhsT=wpj_t[:, cog, gp, :, :],
                        rhs=qa8_t[:, 2 * gp : 2 * gp + 2, tbs],
                        start=(gp == 0),
                        stop=(gp == GP - 1),
                        perf_mode=PM.DoubleRow,
                    )
                nc.vector.scalar_tensor_tensor(
                    out=xT_t[:, cog, tbs],
                    in0=pst,
                    scalar=bproj_t[:, cog : cog + 1],
                    in1=xT_t[:, cog, tbs],
                    op0=OP.add,
                    op1=OP.add,
                )
        if DEBUG_DUMPS:
            nc.sync.dma_start(out=io["dbg_qa8"].ap(), in_=qa8_t[:, :, :])
            nc.sync.dma_start(out=io["dbg_x1"].ap(), in_=xT_t[:, :, :])
        psC_cm.__exit__(None, None, None)

        with (
            tc.tile_pool(name="psD", bufs=4, space="PSUM") as psD,
            tc.tile_pool(name="tmp", bufs=3) as tmpp,
            tc.tile_pool(name="st", bufs=6) as stp,
            tc.tile_pool(name="w1", bufs=2) as w1p,
            tc.tile_pool(name="rows", bufs=4) as rowp,
            tc.tile_pool(name="nrm", bufs=2) as nrmp,
            tc.tile_pool(name="yo", bufs=2) as yop,
        ):
            # ---- silu(t+c) + adaLN (post-phase; PE slack, free DMA ladder) ----
            for g in range(G):
                pt = psD.tile([P, F], FP, name="ptc", tag="ptc", bufs=1)
                nc.tensor.transpose(
                    _r(pt), _r(c_nat[:, g * P : (g + 1) * P]), _r(ident[0:F, 0:F])
                )
                nc.scalar.activation(
                    out=silu_t[:, g, :],
                    in_=pt,
                    func=AF.Silu,
                    bias=t_t[:, g : g + 1],
                    scale=1.0,
                )
            pa_all = psD.tile([P, 16, F], FP, name="pta", tag="pta", bufs=1)
            for ct in range(16):
                wta = wp.tile([P, G, P], BF, name="wta", tag="wa", bufs=3)
                nc.sync.dma_start(
                    out=wta, in_=wada_b.ap()[ct].rearrange("g p c -> p g c")
                )
                for ci in range(G):
                    nc.tensor.matmul(
                        pa_all[:, ct, :],
                        lhsT=wta[:, ci, :],
                        rhs=silu_t[:, ci, :],
                        start=(ci == 0),
                        stop=(ci == G - 1),
                    )
            # fused evictions: shift (+bada), and 1+sc (+bada+1)
            nc.vector.tensor_tensor(
                _r(ada_t[:, 0:8, :]),
                pa_all[:, 0:8, :],
                bada_t[:, 0:8][:, :, None].broadcast_to([P, 8, F]),
                OP.add,
            )
            nc.vector.scalar_tensor_tensor(
                out=_r(ada_t[:, 8:16, :]),
                in0=pa_all[:, 8:16, :],
                scalar=1.0,
                in1=bada_t[:, 8:16][:, :, None].broadcast_to([P, 8, F]),
                op0=OP.add,
                op1=OP.add,
            )
            for tb in range(NTB):
                tbs = slice(tb * TB, (tb + 1) * TB)
                lnab = psD.tile([33, TB], FP, name="lnab", tag="lnab", bufs=2)
                ln_a, ln_b = lnab[0:1, :], lnab[32:33, :]
                for g in range(G):
                    sqt = tmpp.tile([P, TB], BF, tag="tmp")
                    if g % 2 == 0:
                        nc.gpsimd.tensor_mul(sqt, xT_t[:, g, tbs], xT_t[:, g, tbs])
                    else:
                        nc.scalar.activation(
                            out=sqt, in_=xT_t[:, g, tbs], func=AF.Square
                        )
                    nc.tensor.matmul(
                        ln_a,
                        lhsT=ones_bf[:, 0:1],
                        rhs=xT_t[:, g, tbs],
                        start=(g == 0),
                        stop=(g == G - 1),
                    )
                    nc.tensor.matmul(
                        ln_b,
                        lhsT=ones_bf[:, 0:1],
                        rhs=sqt,
                        start=(g == 0),
                        stop=(g == G - 1),
                    )
                mu = stp.tile([1, TB], FP, name="mu", tag="st")
                std = stp.tile([1, TB], FP, name="std", tag="st")
                rst = stp.tile([1, TB], FP, name="rst", tag="st")
                mu_bf = stp.tile([1, TB], BF, name="mubf", tag="st")
                std_bf = stp.tile([1, TB], BF, name="stdbf", tag="st")
                nc.vector.tensor_scalar_mul(_r(mu), ln_a, 1.0 / C)
                nc.vector.tensor_mul(_r(std), mu, mu)
                nc.vector.scalar_tensor_tensor(
                    out=_r(std),
                    in0=ln_b,
                    scalar=1.0 / C,
                    in1=std,
                    op0=OP.mult,
                    op1=OP.subtract,
                )
                nc.scalar.activation(
                    out=_r(std), in_=std, func=AF.Sqrt, bias=eps_t[0:1, :], scale=1.0
                )
                nc.vector.reciprocal(_r(rst), std)
                nc.vector.tensor_copy(out=mu_bf, in_=mu)
                nc.vector.tensor_copy(out=std_bf, in_=std)
                bc32_ps = psD.tile([64, TB], FP, name="bc32", tag="ps")
                nc.tensor.matmul(
                    bc32_ps,
                    lhsT=_r(ones_t[0:1, 0:64]),
                    rhs=_r(rst),
                    start=True,
                    stop=True,
                )
                bc32 = nrmp.tile([64, TB], FP, tag="nrm")
                nc.scalar.copy(out=bc32, in_=bc32_ps)
                y4_ps = psD.tile([64, NT], FP, name="y4", tag="ps")
                for f2 in range(2):
                    f = tb * 2 + f2
                    r32 = slice(32 * f2, 32 * f2 + 32)
                    fcs = slice(f2 * NT, (f2 + 1) * NT)  # cols within tb
                    gcs = slice(tb * TB + f2 * NT, tb * TB + (f2 + 1) * NT)
                    w1 = w1p.tile([P, G, OUTD], BF, tag="w1")
                    nc.gpsimd.tensor_mul(
                        w1,
                        wlin_sb,
                        ada_t[:, 8:16, f : f + 1].broadcast_to([P, G, OUTD]),
                    )
                    ws1_ps = psD.tile([1, OUTD], FP, name="ws1", tag="ps")
                    c2_ps = psD.tile([1, OUTD], FP, name="c2", tag="ps")
                    for g in range(G):
                        nc.tensor.matmul(
                            ws1_ps,
                            lhsT=_r(ada_t[:, 8 + g, f : f + 1]),
                            rhs=_r(wlin_sb[:, g, :]),
                            start=(g == 0),
                            stop=(g == G - 1),
                        )
                        nc.tensor.matmul(
                            c2_ps,
                            lhsT=_r(ada_t[:, g, f : f + 1]),
                            rhs=_r(wlin_sb[:, g, :]),
                            start=(g == 0),
                            stop=(g == G - 1),
                        )
                    ws1n = rowp.tile([1, OUTD], BF, name="ws1n", tag="rows")
                    c2b = rowp.tile([1, OUTD], BF, name="c2b", tag="rows")
                    nc.vector.tensor_scalar_mul(ws1n, ws1_ps, -1.0)
                    nc.vector.tensor_tensor(c2b, c2_ps, blin_row, OP.add)
                    y_ps = y4_ps[r32, :]
                    for g in range(G):
                        nc.tensor.matmul(
                            y_ps,
                            lhsT=w1[:, g, :],
                            rhs=xT_t[:, g, gcs],
                            start=(g == 0),
                            stop=False,
                        )
                    nc.tensor.matmul(
                        y_ps,
                        lhsT=ws1n,
                        rhs=mu_bf[0:1, fcs],
                        start=False,
                        stop=False,
                    )
                    nc.tensor.matmul(
                        y_ps,
                        lhsT=c2b,
                        rhs=std_bf[0:1, fcs],
                        start=False,
                        stop=True,
                    )
                    yt = yop.tile([64, NT], FP, tag="y")
                    nc.vector.tensor_mul(yt[r32, :], y_ps, bc32[r32, fcs])
                    nc.sync.dma_start(out=yT[:, gcs], in_=yt[r32, :])


def declare_io(nc):
    dbg = {}
    if DEBUG_DUMPS:
        dbg = {
            "dbg_q": nc.dram_tensor("dbg_q", [P, G, STOK], BF, kind="ExternalOutput"),
            "dbg_kt": nc.dram_tensor("dbg_kt", [P, G, N2], BF, kind="ExternalOutput"),
            "dbg_vv": nc.dram_tensor("dbg_vv", [KB, P, 8 * SEG], BF, kind="ExternalOutput"),
            "dbg_qa8": nc.dram_tensor("dbg_qa8", [P, G, STOK], F8, kind="ExternalOutput"),
            "dbg_x1": nc.dram_tensor("dbg_x1", [P, G, STOK], BF, kind="ExternalOutput"),
            "dbg_x8": nc.dram_tensor("dbg_x8", [P, G, STOK], F8, kind="ExternalOutput"),
            "dbg_ada": nc.dram_tensor("dbg_ada", [P, 16, F], FP, kind="ExternalOutput"),
        }
    return {
        **dbg,
        "x_sl": nc.dram_tensor("x_sl", [STOK, C], FP, kind="ExternalInput"),
        "v_b": nc.dram_tensor("v_b", [N2, C], FP, kind="ExternalInput"),
        "t_b": nc.dram_tensor("t_b", [C], FP, kind="ExternalInput"),
        "c_sl": nc.dram_tensor("c_sl", [F, C], FP, kind="ExternalInput"),
        "wq8": nc.dram_tensor("wq8", [G, GP, P, 2, P], F8, kind="ExternalInput"),
        "bq": nc.dram_tensor("bq", [C], FP, kind="ExternalInput"),
        "wkvk8": nc.dram_tensor("wkvk8", [G, GP, P, 2, P], F8, kind="ExternalInput"),
        "wkvv8": nc.dram_tensor("wkvv8", [GP, P, 2, C], F8, kind="ExternalInput"),
        "bkv": nc.dram_tensor("bkv", [2 * C], FP, kind="ExternalInput"),
        "wproj8": nc.dram_tensor("wproj8", [G, GP, P, 2, P], F8, kind="ExternalInput"),
        "bproj": nc.dram_tensor("bproj", [C], FP, kind="ExternalInput"),
        "wada_b": nc.dram_tensor("wada_b", [16, G, P, P], BF, kind="ExternalInput"),
        "bada": nc.dram_tensor("bada", [2 * C], FP, kind="ExternalInput"),
        "wlin": nc.dram_tensor("wlin", [C, OUTD], FP, kind="ExternalInput"),
        "blin": nc.dram_tensor("blin", [OUTD], FP, kind="ExternalInput"),
        "yT": nc.dram_tensor("yT", [OUTD, STOK], FP, kind="ExternalOutput"),
    }


def build_nc():
    nc = bacc.Bacc("TRN2", target_bir_lowering=False, debug=False)
    io = declare_io(nc)
    with tile.TileContext(nc) as tc:
        _body(nc, tc, io)
    nc.compile()
    return nc


_CACHE = {}


def _get_nc():
    if "nc" not in _CACHE:
        _CACHE["nc"] = build_nc()
    return _CACHE["nc"]


def _dr_block(w):
    """[C, CO] fp32 -> [CO/128, 4, 128, 2, 128] fp8e5 DoubleRow layout.

    ci = (2*gp + j)*128 + ki  ->  [cog, gp, ki, j, m]"""
    co = w.shape[1] // P
    r = w.reshape(GP, 2, P, co, P)          # [gp, j, ki, cog, m]
    return np.ascontiguousarray(r.transpose(3, 0, 2, 1, 4)).astype(NP8)


def make_in_maps(x, v, t, c, wq, bq, wkv, bkv, wproj, bproj, wada, bada, wlin, blin):
    f32 = lambda a: np.ascontiguousarray(np.asarray(a, dtype=np.float32))
    x, v, t, c = f32(x), f32(v), f32(t), f32(c)
    wq, wkv, wproj, wada = f32(wq), f32(wkv), f32(wproj), f32(wada)

    def blocked_bf(w):  # [cin, cout] -> [co_tile, ci_tile, 128, 128] bf16
        co = w.shape[1] // P
        return np.ascontiguousarray(
            w.reshape(G, P, co, P).transpose(2, 0, 1, 3)
        ).astype(ml_dtypes.bfloat16)

    wkvv = f32(wkv[:, C:])
    wkvv8 = np.ascontiguousarray(
        wkvv.reshape(GP, 2, P, 2 * C // 2).transpose(0, 2, 1, 3)
    ).astype(NP8)  # [gp, ki, j, 1024]

    shared = {
        "wq8": _dr_block(wq),
        "bq": f32(bq),
        "wkvk8": _dr_block(np.ascontiguousarray(wkv[:, :C])),
        "wkvv8": wkvv8,
        "bkv": f32(bkv),
        "wproj8": _dr_block(wproj),
        "bproj": f32(bproj),
        "wada_b": blocked_bf(wada),
        "bada": f32(bada),
        "wlin": f32(wlin),
        "blin": f32(blin),
    }
    in_maps = []
    for m in range(8):
        b, half = divmod(m, 2)
        in_maps.append(
            {
                "x_sl": f32(x[b, half * STOK : (half + 1) * STOK, :]),
                "v_b": f32(v[b]),
                "t_b": f32(t[b]),
                "c_sl": f32(c[b, half * F : (half + 1) * F, :]),
                **shared,
            }
        )
    return in_maps


def assemble_y(results):
    y = np.empty((B, T, NT, OUTD), np.float32)
    for m in range(8):
        b, half = divmod(m, 2)
        yt = np.asarray(results[m]["yT"])  # [OUTD, STOK]
        y[b, half * F : (half + 1) * F] = yt.T.reshape(F, NT, OUTD)
    return y


def kernel(x, v, t, c, wq, bq, wkv, bkv, wproj, bproj, wada, bada, wlin, blin, T=16, H=16):
    nc = _get_nc()
    in_maps = make_in_maps(
        x, v, t, c, wq, bq, wkv, bkv, wproj, bproj, wada, bada, wlin, blin
    )
    res = run_bass_kernel_spmd(nc, in_maps, core_ids=list(range(8)))
    return assemble_y(res.results)


# revision 67
# speedup vs baseline: 1.0922x; 1.0922x over previous
"""Trainium2 Bass kernel for a cross-attention + adaLN-modulated-LN + linear block.

Sharding: 8 cores = 4 batches (B) x 2 token-halves of S=4096.  No collectives:
each core recomputes the (small) kv projection for its batch and processes all
16 attention heads for its 2048 tokens, then LN/modulation/final-linear for its
8 frames.  Host slices inputs per core and reassembles the output.

Device layout is feature-major ("transposed"): activations live as [C, tok]
tiles so every matmul contracts over the SBUF partition dim.

Precision plan (tolerance 2e-2; the residual damps attention-path noise ~40x,
measured end-to-end rel err ~3.9e-3):
  - q / kv / proj projections run in fp8e5m2 with perf_mode=DoubleRow (2
    contraction rows per PE cell -> half the matmul time).  The fp8 rhs
    operands are the natural [128, 2, N] slices of feature-major tiles, so no
    device-side re-layout is needed; weights are host-interleaved.
  - attention operands (kT, q, exp(scores), vv) are bf16; PSUM accumulation
    is fp32 throughout.
  - the LN-stats / final-linear path stays bf16/fp32 (output precision).
Softmax: scores for 2 key-blocks land side-by-side in one 2-bank PSUM tile so
exp runs as one [128,1024] Activation op (the Activation engine is the
attention-phase bottleneck); the softmax denominator rides the attention-output
matmul as an all-ones lhsT column, and each head's 1/denom row is broadcast
across partitions with gpsimd partition_broadcast (NOTE: its output tile must
be base-partition 0; the odd head broadcasts to 128 rows and slices [64:128]).
Scheduling notes: DMAs only on the SP/Act hardware-DGE queues (gpsimd DMA is
software-DGE and stalls the Pool engine); input loads are few and large with
tc.high_priority ordering v -> wkvv -> wkvk -> x -> wq; adaLN runs in the
post-LN phase so its streamed weight loads never block the PE queue; PSUM is
partitioned psT(3)+psV(5) in setup and sc(4)+ao(2)+psC(2) in attention, with
proj and its eviction overlapping attention via the psC pool.
"""

import sys

for _p in ("/opt/trn_rl_repo", "/opt/pypackages"):
    if _p not in sys.path:
        sys.path.append(_p)

import numpy as np
import ml_dtypes

import concourse.bacc as bacc
import concourse.tile as tile
from concourse import mybir
from concourse.bass_utils import run_bass_kernel_spmd
from concourse.masks import make_identity

FP = mybir.dt.float32
FPR = mybir.dt.float32r
BF = mybir.dt.bfloat16
F8 = mybir.dt.float8e5
AF = mybir.ActivationFunctionType
OP = mybir.AluOpType
PM = mybir.MatmulPerfMode
NP8 = ml_dtypes.float8_e5m2


def _r(ap):
    """View an fp32 AP as float32r for full-rate PE matmuls (same bits)."""
    return ap.bitcast(FPR)


# Problem sizes (hardcoded per spec).
B = 4
S = 4096
C = 1024
N2 = 512
H = 16
D = 64
T = 16
NT = 256          # tokens per frame
OUTD = 32

STOK = S // 2     # tokens per core
F = 8             # frames per core
G = C // 128      # 8 channel groups
GP = G // 2       # 4 channel-pair groups (DoubleRow)
TB = 512          # token block (matmul N)
NTB = STOK // TB  # 4
KB = N2 // 128    # 4 key blocks
SCALE = D ** -0.5
EPS = 1e-6
P = 128
SEG = 192         # vv cols per head-pair segment
DEBUG_DUMPS = False


def _body(nc, tc, io):
    with nc.allow_low_precision("bf16/fp8 matmul operands"):
        _body_inner(nc, tc, io)


def _body_inner(nc, tc, io):
    x, v, tvec, cmat = io["x_sl"], io["v_b"], io["t_b"], io["c_sl"]
    wq8, bq = io["wq8"], io["bq"]
    wkvk8, wkvv8, bkv = io["wkvk8"], io["wkvv8"], io["bkv"]
    wproj8, bproj = io["wproj8"], io["bproj"]
    wada_b, bada = io["wada_b"], io["bada"]
    wlin, blin = io["wlin"], io["blin"]
    yT = io["yT"]

    with (
        tc.tile_pool(name="consts", bufs=1) as consts,
        tc.tile_pool(name="xT", bufs=1) as xTp,
        tc.tile_pool(name="x8", bufs=1) as x8p,
        tc.tile_pool(name="qa", bufs=1) as qap,
        tc.tile_pool(name="qa8", bufs=1) as qa8p,
        tc.tile_pool(name="kT", bufs=1) as kTp,
        tc.tile_pool(name="wp", bufs=3) as wp,
        tc.tile_pool(name="wpj", bufs=1) as wpjp,
    ):
        # ---- constants / small inputs ----
        scratch = consts.tile([P, P], FP, tag="scratch")
        make_identity(nc, scratch)
        ident = consts.tile([P, P], FP, tag="ident")
        nc.vector.tensor_copy(out=_r(ident), in_=scratch)
        ones_t = consts.tile([P, P], FP, tag="ones")
        nc.vector.tensor_scalar(
            _r(ones_t), scratch, 0.0, 1.0, op0=OP.mult, op1=OP.add
        )
        ones_bf = consts.tile([P, P], BF, tag="onesbf")
        nc.vector.tensor_copy(out=ones_bf, in_=ones_t)
        ident_bf = consts.tile([P, P], BF, tag="identbf")
        nc.vector.tensor_copy(out=ident_bf, in_=scratch)
        eps_t = consts.tile([P, 1], FP, tag="eps")
        nc.vector.memset(eps_t, EPS)

        bq_t = consts.tile([P, G], FP, tag="bq")
        nc.sync.dma_start(out=bq_t, in_=bq.ap().rearrange("(g p) -> p g", p=P))
        bkvk_t = consts.tile([P, G], FP, tag="bkvk")
        nc.sync.dma_start(
            out=bkvk_t, in_=bkv.ap()[0:C].rearrange("(g p) -> p g", p=P)
        )
        bproj_t = consts.tile([P, G], FP, tag="bproj")
        nc.sync.dma_start(out=bproj_t, in_=bproj.ap().rearrange("(g p) -> p g", p=P))
        bada_t = consts.tile([P, 16], FP, tag="bada")
        nc.sync.dma_start(out=bada_t, in_=bada.ap().rearrange("(g p) -> p g", p=P))
        blin_row = consts.tile([1, OUTD], FP, tag="blin")
        nc.sync.dma_start(
            out=blin_row, in_=blin.ap().rearrange("(one o) -> one o", one=1)
        )
        t_t = consts.tile([P, G], FP, tag="tvec")
        nc.sync.dma_start(out=t_t, in_=tvec.ap().rearrange("(g p) -> p g", p=P))
        wlin_sb = consts.tile([P, G, OUTD], FP, tag="wlin")
        nc.sync.dma_start(
            out=_r(wlin_sb), in_=_r(wlin.ap().rearrange("(ci p) o -> p ci o", p=P))
        )
        silu_t = consts.tile([P, G, F], BF, tag="silu")
        ada_t = consts.tile([P, 16, F], FP, tag="ada")
        bkvv_row = consts.tile([1, C], FP, tag="bkvv")
        nc.sync.dma_start(
            out=_r(bkvv_row),
            in_=_r(bkv.ap()[C : 2 * C].rearrange("(one n) -> one n", one=1)),
        )
        c_nat = consts.tile([F, C], FP, tag="cnat")
        nc.sync.dma_start(out=_r(c_nat), in_=_r(cmat[:, :]))

        # ---- persistent activation buffers ----
        xT_t = xTp.tile([P, G, STOK], BF, tag="xT")       # x then x1 (bf16)
        xT8_t = x8p.tile([P, G, STOK], F8, tag="xT8")     # fp8 copy for q-proj
        qa_t = qap.tile([P, G, STOK], BF, tag="qa")       # q (bf16)
        qa8_t = qa8p.tile([P, G, STOK], F8, tag="qa8")    # normalized attn out
        kt_t = kTp.tile([P, G, N2], BF, tag="kT")         # k (bf16)
        # proj weights stay resident (fp8: 8KB/partition); DMA issued late
        wpj_t = wpjp.tile([P, G, GP, 2, P], F8, tag="wpj")

        # vv: key-major value matrix (bf16). Per head-pair g, a SEG=192-col
        # segment: [0:64] even-head data, [64] ones column (denominator),
        # [65:128] junk, [128:192] odd-head data.  Even lhsT window cols 0:65
        # -> rows 0:64 data + row 64 denom; odd window cols 64:192 -> row 0
        # denom + rows 64:128 data.
        with tc.tile_pool(name="vv", bufs=1) as vvp:
            vv = [
                vvp.tile([P, 8 * SEG], BF, name=f"vv{kb}", tag=f"vv{kb}")
                for kb in range(KB)
            ]
            for kb in range(KB):
                ones_cols = vv[kb].rearrange("p (a r) -> p a r", r=SEG)[:, :, 64:65]
                nc.vector.memset(ones_cols, 1.0)

            with (
                tc.tile_pool(name="psT", bufs=3, space="PSUM") as psT,
                tc.tile_pool(name="psV", bufs=5, space="PSUM") as psV,
            ):
                with (
                    tc.tile_pool(name="vT", bufs=1) as vTp,
                    tc.tile_pool(name="ld", bufs=4) as ldp,
                ):
                    vT8_t = vTp.tile([P, G, N2], F8, tag="vT8")

                    # ---- transpose v (evict straight to fp8) ----
                    for kt in range(KB):
                        v_nat = ldp.tile([P, C], BF, name="vn", tag="ldv", bufs=2)
                        with tc.high_priority(offset=100000):
                            for vh in range(2):
                                (nc.sync, nc.scalar)[vh].dma_start(
                                    out=v_nat[:, vh * TB : (vh + 1) * TB],
                                    in_=v[kt * P : (kt + 1) * P, vh * TB : (vh + 1) * TB],
                                )
                        for g4 in range(2):
                            pt = psT.tile([P, 4 * P], BF, name="ptv", tag="ps")
                            for j in range(4):
                                jc = g4 * TB + j * P
                                nc.tensor.transpose(
                                    pt[:, j * P : (j + 1) * P],
                                    v_nat[:, jc : jc + P],
                                    ident_bf,
                                )
                            nc.any.tensor_copy(
                                out=vT8_t[:, g4 * 4 : g4 * 4 + 4, kt * P : (kt + 1) * P],
                                in_=pt.rearrange("p (a c) -> p a c", c=P),
                            )

                    # ---- transpose x (evict bf16 + fp8 via Act from psum) ----
                    for tt2 in range(STOK // (2 * P)):
                        x_nat = ldp.tile([P, 2, C], BF, name="xn", tag="ldx", bufs=2)
                        dma_eng = (nc.sync, nc.scalar)[tt2 % 2]
                        with tc.high_priority(offset=98000):
                            dma_eng.dma_start(
                                out=x_nat,
                                in_=x[2 * tt2 * P : (2 * tt2 + 2) * P, :].rearrange(
                                    "(u p) c -> p u c", p=P
                                ),
                            )
                        for u in range(2):
                            tt = 2 * tt2 + u
                            for g4 in range(2):
                                pt = psT.tile([P, 4 * P], BF, name="ptx", tag="ps")
                                for j in range(4):
                                    jc = g4 * TB + j * P
                                    nc.tensor.transpose(
                                        pt[:, j * P : (j + 1) * P],
                                        x_nat[:, u, jc : jc + P],
                                        ident_bf,
                                    )
                                gsl = slice(g4 * 4, g4 * 4 + 4)
                                tsl = slice(tt * P, (tt + 1) * P)
                                if (tt + g4) % 2 == 0:
                                    nc.vector.tensor_copy(
                                        out=xT_t[:, gsl, tsl],
                                        in_=pt.rearrange("p (a c) -> p a c", c=P),
                                    )
                                    nc.scalar.copy(
                                        out=xT8_t[:, gsl, tsl],
                                        in_=pt.rearrange("p (a c) -> p a c", c=P),
                                    )
                                else:
                                    nc.scalar.copy(
                                        out=xT_t[:, gsl, tsl],
                                        in_=pt.rearrange("p (a c) -> p a c", c=P),
                                    )
                                    nc.vector.tensor_copy(
                                        out=xT8_t[:, gsl, tsl],
                                        in_=pt.rearrange("p (a c) -> p a c", c=P),
                                    )

                    # ---- vv build (fp8 DoubleRow), bias folded via ones-row MM ----
                    wvv_sb = ldp.tile([P, GP, 2, C], F8, name="wvvsb", tag="wv8", bufs=1)
                    with tc.high_priority(offset=99000):
                        nc.scalar.dma_start(
                            out=wvv_sb, in_=wkvv8.ap().rearrange("gp p j c -> p gp j c")
                        )
                    for half in range(2):
                        pss = [
                            psV.tile([P, TB], FP, name="psv", tag="ps")
                            for _ in range(KB)
                        ]
                        for kb in range(KB):
                            # bias first (fp32r), then fp8 DR accumulation
                            nc.tensor.matmul(
                                pss[kb],
                                lhsT=_r(ones_t[0:1, 0:P]),
                                rhs=_r(bkvv_row[0:1, half * TB : (half + 1) * TB]),
                                start=True,
                                stop=False,
                            )
                            for gp in range(GP):
                                nc.tensor.matmul(
                                    pss[kb],
                                    lhsT=vT8_t[:, 2 * gp : 2 * gp + 2, kb * P : (kb + 1) * P],
                                    rhs=wvv_sb[:, gp, :, half * TB : (half + 1) * TB],
                                    start=False,
                                    stop=(gp == GP - 1),
                                    perf_mode=PM.DoubleRow,
                                )
                        for kb in range(KB):
                            vvr = vv[kb].rearrange("p (a r) -> p a r", r=SEG)
                            src = pss[kb].rearrange("p (a q j) -> p a q j", q=2, j=64)
                            gs = slice(half * 4, half * 4 + 4)
                            eng = nc.vector if kb % 2 == 0 else nc.scalar
                            if kb % 2 == 0:
                                nc.vector.tensor_copy(
                                    out=vvr[:, gs, 0:64], in_=src[:, :, 0, :]
                                )
                                nc.vector.tensor_copy(
                                    out=vvr[:, gs, 128:192], in_=src[:, :, 1, :]
                                )
                            else:
                                nc.scalar.copy(out=vvr[:, gs, 0:64], in_=src[:, :, 0, :])
                                nc.scalar.copy(
                                    out=vvr[:, gs, 128:192], in_=src[:, :, 1, :]
                                )

                    # ---- kT (fp8 DoubleRow; frees vT afterwards) ----
                    wkv_sb = ldp.tile([P, G, GP, 2, P], F8, name="wkvsb", tag="wkvsb", bufs=1)
                    with tc.high_priority(offset=98500):
                        nc.scalar.dma_start(
                            out=wkv_sb,
                            in_=wkvk8.ap().rearrange("g gp p j m -> p g gp j m"),
                        )
                    for g in range(G):
                        psk = psV.tile([P, N2], FP, name="psk", tag="ps")
                        for gp in range(GP):
                            nc.tensor.matmul(
                                psk,
                                lhsT=wkv_sb[:, g, gp, :, :],
                                rhs=vT8_t[:, 2 * gp : 2 * gp + 2, :],
                                start=(gp == 0),
                                stop=(gp == GP - 1),
                                perf_mode=PM.DoubleRow,
                            )
                        nc.vector.tensor_scalar_add(
                            kt_t[:, g, :], psk, bkvk_t[:, g : g + 1]
                        )

                    # late prefetch of proj weights
                    nc.scalar.dma_start(
                        out=wpj_t,
                        in_=wproj8.ap().rearrange("cog gp p j m -> p cog gp j m"),
                    )


                # ---- q projection (fp8 DoubleRow; tb-major) ----
                wq_sb = wp.tile([P, G, GP, 2, P], F8, name="wqsb", tag="wqsb", bufs=1)
                with tc.high_priority(offset=97500):
                    nc.sync.dma_start(
                        out=wq_sb,
                        in_=wq8.ap().rearrange("cog gp p j m -> p cog gp j m"),
                    )
                for tb in range(NTB):
                    for cog in range(G):
                        pst = psV.tile([P, TB], FP, name="psq", tag="ps")
                        for gp in range(GP):
                            nc.tensor.matmul(
                                pst,
                                lhsT=wq_sb[:, cog, gp, :, :],
                                rhs=xT8_t[:, 2 * gp : 2 * gp + 2, tb * TB : (tb + 1) * TB],
                                start=(gp == 0),
                                stop=(gp == GP - 1),
                                perf_mode=PM.DoubleRow,
                            )
                        if (cog + tb) % 2 == 0:
                            nc.vector.tensor_scalar_add(
                                qa_t[:, cog, tb * TB : (tb + 1) * TB],
                                pst,
                                bq_t[:, cog : cog + 1],
                            )
                        else:
                            nc.scalar.activation(
                                out=qa_t[:, cog, tb * TB : (tb + 1) * TB],
                                in_=pst,
                                func=AF.Identity,
                                bias=bq_t[:, cog : cog + 1],
                                scale=1.0,
                            )

            # ---- attention (per head-pair; pipelined over (tb, g)) ----
            psC_cm = tc.tile_pool(name="psC", bufs=2, space="PSUM")
            psC = psC_cm.__enter__()
            with (
                tc.tile_pool(name="sc", bufs=2, space="PSUM") as scp,
                tc.tile_pool(name="ao", bufs=2, space="PSUM") as aop,
                tc.tile_pool(name="exp", bufs=6) as expp,
                tc.tile_pool(name="dn", bufs=2) as dnp,
            ):
                for tb in range(NTB):
                    tbs = slice(tb * TB, (tb + 1) * TB)
                    for g in range(G):
                        ao_tiles = [None, None]
                        rcps = [
                            dnp.tile([1, TB], BF, name=f"rcp{h2}", tag=f"rcp{h2}", bufs=3)
                            for h2 in range(2)
                        ]
                        for half in range(2):
                            r0 = half * 64
                            dr = 64 - 64 * half  # denom row: 64 (even), 0 (odd)
                            if half == 0:
                                lhs_lo, lhs_hi = 0, 65
                                ao_ps = aop.tile([65, TB], FP, name="aoe", tag="ao")
                            else:
                                lhs_lo, lhs_hi = 64, SEG
                                ao_ps = aop.tile([P, TB], FP, name="aoo", tag="ao")
                            ao_tiles[half] = ao_ps
                            exs = []
                            for kk in range(2):  # two fused (2-bank) score tiles
                                sc_ps = scp.tile([P, 2 * TB], FP, name="scs", tag="sc")
                                for kb2 in range(2):
                                    kb = 2 * kk + kb2
                                    nc.tensor.matmul(
                                        sc_ps[:, kb2 * TB : (kb2 + 1) * TB],
                                        lhsT=kt_t[r0 : r0 + 64, g, kb * P : (kb + 1) * P],
                                        rhs=qa_t[r0 : r0 + 64, g, tbs],
                                        start=True,
                                        stop=True,
                                    )
                                ex = expp.tile([P, 2 * TB], BF, tag="e")
                                nc.scalar.activation(
                                    out=ex, in_=sc_ps, func=AF.Exp, scale=SCALE
                                )
                                exs.append(ex)
                            for kb in range(KB):
                                nc.tensor.matmul(
                                    ao_ps,
                                    lhsT=vv[kb][
                                        :, g * SEG + lhs_lo : g * SEG + lhs_hi
                                    ],
                                    rhs=exs[kb // 2][:, (kb % 2) * TB : (kb % 2 + 1) * TB],
                                    start=(kb == 0),
                                    stop=(kb == KB - 1),
                                )
                            nc.vector.reciprocal(
                                out=rcps[half], in_=ao_ps[dr : dr + 1, :]
                            )
                        # broadcast each head's 1/denom on GpSimd.  NOTE:
                        # partition_broadcast requires a base-0 output tile;
                        # the odd head broadcasts to all 128 rows and the mul
                        # reads the base-64 slice.
                        dnb_e = dnp.tile([64, TB], BF, name="dnbe", tag="dne")
                        dnb_o = dnp.tile([P, TB], BF, name="dnbo", tag="dno")
                        nc.gpsimd.partition_broadcast(dnb_e, rcps[0])
                        nc.gpsimd.partition_broadcast(dnb_o, rcps[1])
                        nc.vector.tensor_mul(
                            qa8_t[0:64, g, tbs],
                            ao_tiles[0][0:64, :],
                            dnb_e,
                        )
                        nc.vector.tensor_mul(
                            qa8_t[64:128, g, tbs],
                            ao_tiles[1][64:128, :],
                            dnb_o[64:128, :],
                        )

        # ---- proj + residual (fp8 DoubleRow), then LN stats (shared psum) ----
        for tb in range(NTB):
            tbs = slice(tb * TB, (tb + 1) * TB)
            for cog in range(G):
                pst = psC.tile([P, TB], FP, name="psp", tag="ps", bufs=2)
                for gp in range(GP):
                    nc.tensor.matmul(
                        pst,
                        lhsT=wpj_t[:, cog, gp, :, :],
                        rhs=qa8_t[:, 2 * gp : 2 * gp + 2, tbs],
                        start=(gp == 0),
                        stop=(gp == GP - 1),
                        perf_mode=PM.DoubleRow,
                    )
                nc.vector.scalar_tensor_tensor(
                    out=xT_t[:, cog, tbs],
                    in0=pst,
                    scalar=bproj_t[:, cog : cog + 1],
                    in1=xT_t[:, cog, tbs],
                    op0=OP.add,
                    op1=OP.add,
                )
        if DEBUG_DUMPS:
            nc.sync.dma_start(out=io["dbg_qa8"].ap(), in_=qa8_t[:, :, :])
            nc.sync.dma_start(out=io["dbg_x1"].ap(), in_=xT_t[:, :, :])
        psC_cm.__exit__(None, None, None)

        with (
            tc.tile_pool(name="psD", bufs=4, space="PSUM") as psD,
            tc.tile_pool(name="tmp", bufs=3) as tmpp,
            tc.tile_pool(name="st", bufs=6) as stp,
            tc.tile_pool(name="w1", bufs=2) as w1p,
            tc.tile_pool(name="rows", bufs=4) as rowp,
            tc.tile_pool(name="nrm", bufs=2) as nrmp,
            tc.tile_pool(name="yo", bufs=2) as yop,
        ):
            # ---- silu(t+c) + adaLN (post-phase; PE slack, free DMA ladder) ----
            for g in range(G):
                pt = psD.tile([P, F], FP, name="ptc", tag="ptc", bufs=1)
                nc.tensor.transpose(
                    _r(pt), _r(c_nat[:, g * P : (g + 1) * P]), _r(ident[0:F, 0:F])
                )
                nc.scalar.activation(
                    out=silu_t[:, g, :],
                    in_=pt,
                    func=AF.Silu,
                    bias=t_t[:, g : g + 1],
                    scale=1.0,
                )
            pa_all = psD.tile([P, 16, F], FP, name="pta", tag="pta", bufs=1)
            for ct in range(16):
                wta = wp.tile([P, G, P], BF, name="wta", tag="wa", bufs=3)
                nc.sync.dma_start(
                    out=wta, in_=wada_b.ap()[ct].rearrange("g p c -> p g c")
                )
                for ci in range(G):
                    nc.tensor.matmul(
                        pa_all[:, ct, :],
                        lhsT=wta[:, ci, :],
                        rhs=silu_t[:, ci, :],
                        start=(ci == 0),
                        stop=(ci == G - 1),
                    )
            # fused evictions: shift (+bada), and 1+sc (+bada+1)
            nc.vector.tensor_tensor(
                _r(ada_t[:, 0:8, :]),
                pa_all[:, 0:8, :],
                bada_t[:, 0:8][:, :, None].broadcast_to([P, 8, F]),
                OP.add,
            )
            nc.vector.scalar_tensor_tensor(
                out=_r(ada_t[:, 8:16, :]),
                in0=pa_all[:, 8:16, :],
                scalar=1.0,
                in1=bada_t[:, 8:16][:, :, None].broadcast_to([P, 8, F]),
                op0=OP.add,
                op1=OP.add,
            )
            for tb in range(NTB):
                tbs = slice(tb * TB, (tb + 1) * TB)
                lnab = psD.tile([33, TB], FP, name="lnab", tag="lnab", bufs=2)
                ln_a, ln_b = lnab[0:1, :], lnab[32:33, :]
                for g in range(G):
                    sqt = tmpp.tile([P, TB], BF, tag="tmp")
                    if g % 2 == 0:
                        nc.gpsimd.tensor_mul(sqt, xT_t[:, g, tbs], xT_t[:, g, tbs])
                    else:
                        nc.scalar.activation(
                            out=sqt, in_=xT_t[:, g, tbs], func=AF.Square
                        )
                    nc.tensor.matmul(
                        ln_a,
                        lhsT=ones_bf[:, 0:1],
                        rhs=xT_t[:, g, tbs],
                        start=(g == 0),
                        stop=(g == G - 1),
                    )
                    nc.tensor.matmul(
                        ln_b,
                        lhsT=ones_bf[:, 0:1],
                        rhs=sqt,
                        start=(g == 0),
                        stop=(g == G - 1),
                    )
                mu = stp.tile([1, TB], FP, name="mu", tag="st")
                std = stp.tile([1, TB], FP, name="std", tag="st")
                rst = stp.tile([1, TB], FP, name="rst", tag="st")
                mu_bf = stp.tile([1, TB], BF, name="mubf", tag="st")
                std_bf = stp.tile([1, TB], BF, name="stdbf", tag="st")
                nc.vector.tensor_scalar_mul(_r(mu), ln_a, 1.0 / C)
                nc.vector.tensor_mul(_r(std), mu, mu)
                nc.vector.scalar_tensor_tensor(
                    out=_r(std),
                    in0=ln_b,
                    scalar=1.0 / C,
                    in1=std,
                    op0=OP.mult,
                    op1=OP.subtract,
                )
                nc.scalar.activation(
                    out=_r(std), in_=std, func=AF.Sqrt, bias=eps_t[0:1, :], scale=1.0
                )
                nc.vector.reciprocal(_r(rst), std)
                nc.vector.tensor_copy(out=mu_bf, in_=mu)
                nc.vector.tensor_copy(out=std_bf, in_=std)
                bc32_ps = psD.tile([64, TB], FP, name="bc32", tag="ps")
                nc.tensor.matmul(
                    bc32_ps,
                    lhsT=_r(ones_t[0:1, 0:64]),
                    rhs=_r(rst),
                    start=True,
                    stop=True,
                )
                bc32 = nrmp.tile([64, TB], FP, tag="nrm")
                nc.scalar.copy(out=bc32, in_=bc32_ps)
                y4_ps = psD.tile([64, NT], FP, name="y4", tag="ps")
                for f2 in range(2):
                    f = tb * 2 + f2
                    r32 = slice(32 * f2, 32 * f2 + 32)
                    fcs = slice(f2 * NT, (f2 + 1) * NT)  # cols within tb
                    gcs = slice(tb * TB + f2 * NT, tb * TB + (f2 + 1) * NT)
                    w1 = w1p.tile([P, G, OUTD], BF, tag="w1")
                    nc.gpsimd.tensor_mul(
                        w1,
                        wlin_sb,
                        ada_t[:, 8:16, f : f + 1].broadcast_to([P, G, OUTD]),
                    )
                    ws1_ps = psD.tile([1, OUTD], FP, name="ws1", tag="ps")
                    c2_ps = psD.tile([1, OUTD], FP, name="c2", tag="ps")
                    for g in range(G):
                        nc.tensor.matmul(
                            ws1_ps,
                            lhsT=_r(ada_t[:, 8 + g, f : f + 1]),
                            rhs=_r(wlin_sb[:, g, :]),
                            start=(g == 0),
                            stop=(g == G - 1),
                        )
                        nc.tensor.matmul(
                            c2_ps,
                            lhsT=_r(ada_t[:, g, f : f + 1]),
                            rhs=_r(wlin_sb[:, g, :]),
                            start=(g == 0),
                            stop=(g == G - 1),
                        )
                    ws1n = rowp.tile([1, OUTD], BF, name="ws1n", tag="rows")
                    c2b = rowp.tile([1, OUTD], BF, name="c2b", tag="rows")
                    nc.vector.tensor_scalar_mul(ws1n, ws1_ps, -1.0)
                    nc.vector.tensor_tensor(c2b, c2_ps, blin_row, OP.add)
                    y_ps = y4_ps[r32, :]
                    for g in range(G):
                        nc.tensor.matmul(
                            y_ps,
                            lhsT=w1[:, g, :],
                            rhs=xT_t[:, g, gcs],
                            start=(g == 0),
                            stop=False,
                        )
                    nc.tensor.matmul(
                        y_ps,
                        lhsT=ws1n,
                        rhs=mu_bf[0:1, fcs],
                        start=False,
                        stop=False,
                    )
                    nc.tensor.matmul(
                        y_ps,
                        lhsT=c2b,
                        rhs=std_bf[0:1, fcs],
                        start=False,
                        stop=True,
                    )
                    yt = yop.tile([64, NT], FP, tag="y")
                    nc.vector.tensor_mul(yt[r32, :], y_ps, bc32[r32, fcs])
                    nc.sync.dma_start(out=yT[:, gcs], in_=yt[r32, :])


def declare_io(nc):
    dbg = {}
    if DEBUG_DUMPS:
        dbg = {
            "dbg_q": nc.dram_tensor("dbg_q", [P, G, STOK], BF, kind="ExternalOutput"),
            "dbg_kt": nc.dram_tensor("dbg_kt", [P, G, N2], BF, kind="ExternalOutput"),
            "dbg_vv": nc.dram_tensor("dbg_vv", [KB, P, 8 * SEG], BF, kind="ExternalOutput"),
            "dbg_qa8": nc.dram_tensor("dbg_qa8", [P, G, STOK], F8, kind="ExternalOutput"),
            "dbg_x1": nc.dram_tensor("dbg_x1", [P, G, STOK], BF, kind="ExternalOutput"),
            "dbg_x8": nc.dram_tensor("dbg_x8", [P, G, STOK], F8, kind="ExternalOutput"),
            "dbg_ada": nc.dram_tensor("dbg_ada", [P, 16, F], FP, kind="ExternalOutput"),
        }
    return {
        **dbg,
        "x_sl": nc.dram_tensor("x_sl", [STOK, C], BF, kind="ExternalInput"),
        "v_b": nc.dram_tensor("v_b", [N2, C], BF, kind="ExternalInput"),
        "t_b": nc.dram_tensor("t_b", [C], FP, kind="ExternalInput"),
        "c_sl": nc.dram_tensor("c_sl", [F, C], FP, kind="ExternalInput"),
        "wq8": nc.dram_tensor("wq8", [G, GP, P, 2, P], F8, kind="ExternalInput"),
        "bq": nc.dram_tensor("bq", [C], FP, kind="ExternalInput"),
        "wkvk8": nc.dram_tensor("wkvk8", [G, GP, P, 2, P], F8, kind="ExternalInput"),
        "wkvv8": nc.dram_tensor("wkvv8", [GP, P, 2, C], F8, kind="ExternalInput"),
        "bkv": nc.dram_tensor("bkv", [2 * C], FP, kind="ExternalInput"),
        "wproj8": nc.dram_tensor("wproj8", [G, GP, P, 2, P], F8, kind="ExternalInput"),
        "bproj": nc.dram_tensor("bproj", [C], FP, kind="ExternalInput"),
        "wada_b": nc.dram_tensor("wada_b", [16, G, P, P], BF, kind="ExternalInput"),
        "bada": nc.dram_tensor("bada", [2 * C], FP, kind="ExternalInput"),
        "wlin": nc.dram_tensor("wlin", [C, OUTD], FP, kind="ExternalInput"),
        "blin": nc.dram_tensor("blin", [OUTD], FP, kind="ExternalInput"),
        "yT": nc.dram_tensor("yT", [OUTD, STOK], FP, kind="ExternalOutput"),
    }


def build_nc():
    nc = bacc.Bacc("TRN2", target_bir_lowering=False, debug=False)
    io = declare_io(nc)
    with tile.TileContext(nc) as tc:
        _body(nc, tc, io)
    nc.compile()
    return nc


_CACHE = {}


def _get_nc():
    if "nc" not in _CACHE:
        _CACHE["nc"] = build_nc()
    return _CACHE["nc"]


def _dr_block(w):
    """[C, CO] fp32 -> [CO/128, 4, 128, 2, 128] fp8e5 DoubleRow layout.

    ci = (2*gp + j)*128 + ki  ->  [cog, gp, ki, j, m]"""
    co = w.shape[1] // P
    r = w.reshape(GP, 2, P, co, P)          # [gp, j, ki, cog, m]
    return np.ascontiguousarray(r.transpose(3, 0, 2, 1, 4)).astype(NP8)


def make_in_maps(x, v, t, c, wq, bq, wkv, bkv, wproj, bproj, wada, bada, wlin, blin):
    f32 = lambda a: np.ascontiguousarray(np.asarray(a, dtype=np.float32))
    x, v, t, c = f32(x), f32(v), f32(t), f32(c)
    wq, wkv, wproj, wada = f32(wq), f32(wkv), f32(wproj), f32(wada)

    def blocked_bf(w):  # [cin, cout] -> [co_tile, ci_tile, 128, 128] bf16
        co = w.shape[1] // P
        return np.ascontiguousarray(
            w.reshape(G, P, co, P).transpose(2, 0, 1, 3)
        ).astype(ml_dtypes.bfloat16)

    wkvv = f32(wkv[:, C:])
    wkvv8 = np.ascontiguousarray(
        wkvv.reshape(GP, 2, P, 2 * C // 2).transpose(0, 2, 1, 3)
    ).astype(NP8)  # [gp, ki, j, 1024]

    shared = {
        "wq8": _dr_block(wq),
        "bq": f32(bq),
        "wkvk8": _dr_block(np.ascontiguousarray(wkv[:, :C])),
        "wkvv8": wkvv8,
        "bkv": f32(bkv),
        "wproj8": _dr_block(wproj),
        "bproj": f32(bproj),
        "wada_b": blocked_bf(wada),
        "bada": f32(bada),
        "wlin": f32(wlin),
        "blin": f32(blin),
    }
    in_maps = []
    for m in range(8):
        b, half = divmod(m, 2)
        in_maps.append(
            {
                "x_sl": np.ascontiguousarray(
                    x[b, half * STOK : (half + 1) * STOK, :]
                ).astype(ml_dtypes.bfloat16),
                "v_b": np.ascontiguousarray(v[b]).astype(ml_dtypes.bfloat16),
                "t_b": f32(t[b]),
                "c_sl": f32(c[b, half * F : (half + 1) * F, :]),
                **shared,
            }
        )
    return in_maps


def assemble_y(results):
    y = np.empty((B, T, NT, OUTD), np.float32)
    for m in range(8):
        b, half = divmod(m, 2)
        yt = np.asarray(results[m]["yT"])  # [OUTD, STOK]
        y[b, half * F : (half + 1) * F] = yt.T.reshape(F, NT, OUTD)
    return y


def kernel(x, v, t, c, wq, bq, wkv, bkv, wproj, bproj, wada, bada, wlin, blin, T=16, H=16):
    nc = _get_nc()
    in_maps = make_in_maps(
        x, v, t, c, wq, bq, wkv, bkv, wproj, bproj, wada, bada, wlin, blin
    )
    res = run_bass_kernel_spmd(nc, in_maps, core_ids=list(range(8)))
    return assemble_y(res.results)
